# revision 30
# baseline (speedup 1.0000x reference)
"""Trainium2 Bass kernel for a dense transformer block (attention + MLP, 2 LayerNorms).

Sharding: data-parallel over 8 cores with a balanced-causal split: the two
cores of a batch own query blocks {0,3,4,7} and {1,2,5,6} respectively
(context work per core is equal), each recomputing full-context K/V locally,
so no collectives are needed. A shared static per-context-tile query-column
start (QSTART) bounds the computed score region; one 128-col mask block per
context tile (tri/zero/ones, per-core data) finishes causality.

Everything on the matmul path is bf16 (the PE is N-column-bound, so bf16
matches fp32r peak on big tiles and is 4x faster on N=128 tiles, while
halving DMA bytes and doubling DVE throughput). PSUM accumulation is fp32.
Softmax denominators ride the AV matmul as an appended ones-column of V,
are staged per-head on 16 partitions, reciprocated in one DVE op, and
broadcast via a k=16 selector matmul. LayerNorm stats accumulate via
ones-column matmuls interleaved with the producer evictions; row math runs
on partition-0 rows (DVE/ACT cost is free-size bound) using
var^-0.25 = exp(-ln(var)/4) so Exp/Ln/Relu share one ACT table; mean/rstd
broadcast to all partitions with a 1-row PE matmul. The output is stored
feature-major bf16 and transposed on the host.
"""

from contextlib import ExitStack

import ml_dtypes
import numpy as np

import concourse.bacc as bacc
import concourse.bass as bass
import concourse.tile as tile
from concourse import mybir
from concourse.bass_utils import run_bass_kernel_spmd

B, S, D, H = 4, 1024, 1024, 16
DH = D // H
EPS = 1e-5
TOK = 512   # queries per core
CTX = 1024  # context tokens per core
P = 128
F32 = mybir.dt.float32
F32R = mybir.dt.float32r
BF16 = mybir.dt.bfloat16
AF = mybir.ActivationFunctionType
OP = mybir.AluOpType

N_CORES = 8
DEBUG = False
QSTART = [0, 0, 0, 0, 0, 128, 256, 384]   # first query col computed per ctx tile
TT_ORDER = [4, 5, 6, 7, 0, 1, 2, 3]       # own-token tiles first (early PE work)


def _r(ap):
    """View an fp32 AP as float32r for full-rate PE matmuls."""
    return ap.bitcast(F32R)


def build_block_kernel(nc, tc, io):
    ctx = ExitStack()
    (xt, wq_all_h, wk_all_h, wv3, b_qkv, wat_all_h, b_attn, ln1_g, ln1_b,
     wfc4, b_fc, wmlp4, b_mlp, ln2_g, ln2_b, maskb_h, vcol_h, bvA_h, bvB_h,
     sel16_h, xo_h, out, dbg_a, dbg_dd, dbg_st, dbg_r1, dbg_h1) = io

    const = ctx.enter_context(tc.tile_pool(name="const", bufs=1))

    ones_row = const.tile([1, P], F32)       # lhsT for row->all-partition bcast
    nc.vector.memset(ones_row, 1.0)
    ones_bf = const.tile([P, 1], BF16)       # lhsT for column-sum stats
    nc.vector.memset(ones_bf, 1.0)
    sel16 = const.tile([H, 8, P], BF16)      # per-head-pair selector lhsT
    eps_c = const.tile([P, 1], F32)
    nc.vector.memset(eps_c, EPS)

    tri = const.tile([P, P], BF16)           # tri[m, n] = n >= m
    vcol = const.tile([P, 8 * H], BF16)      # per-core denominator column
    bvA = const.tile([P, D], BF16)           # v-bias for ctx tiles 0..3
    bvB = const.tile([P, D], BF16)           # v-bias for ctx tiles 4..7

    def load_consts():
        nc.sync.dma_start(out=tri, in_=tri_h)
        nc.sync.dma_start(out=vcol, in_=vcol_h)
        for bv_t, bv_src in ((bvA, bvA_h), (bvB, bvB_h)):
            nc.sync.dma_start(
                out=bv_t,
                in_=bass.AP(tensor=bv_src.tensor, offset=bv_src.offset,
                            ap=[[0, P]] + list(bv_src.ap)))

    def col_param(src_ap, n_tiles, name):
        t = const.tile([P, n_tiles], F32, name=name)
        nc.sync.dma_start(out=t, in_=src_ap.rearrange("(t p) -> p t", p=P))
        return t

    params = {}

    def load_params():
        params["bq_s"] = col_param(b_qkv[0:D], 8, "bq_s")
        bq_sc = const.tile([P, 8], F32)
        nc.vector.tensor_scalar_mul(out=bq_sc, in0=params["bq_s"],
                                    scalar1=float(1.0 / np.sqrt(DH)))
        params["bq_sc"] = bq_sc
        params["bk_s"] = col_param(b_qkv[D:2 * D], 8, "bk_s")
        params["battn_s"] = col_param(b_attn, 8, "battn_s")
        params["ln1g_s"] = col_param(ln1_g, 8, "ln1g_s")
        params["ln1b_s"] = col_param(ln1_b, 8, "ln1b_s")
        params["bfc_s"] = col_param(b_fc, 32, "bfc_s")
        params["bmlp_s"] = col_param(b_mlp, 8, "bmlp_s")
        params["ln2g_s"] = col_param(ln2_g, 8, "ln2g_s")
        params["ln2b_s"] = col_param(ln2_b, 8, "ln2b_s")
        nc.sync.dma_start(out=sel16, in_=sel16_h)

    ps_big = ctx.enter_context(tc.tile_pool(name="ps_big", bufs=4, space="PSUM"))

    xa_pool = ctx.enter_context(tc.tile_pool(name="xa_pool", bufs=1))
    X_f = xa_pool.tile([P, 8, CTX], BF16)        # x^T, feature-major

    a_pool = ctx.enter_context(tc.tile_pool(name="a_pool", bufs=1))
    a_all = a_pool.tile([P, 8, TOK], BF16)       # attention out^T per head-pair

    att_stack = ExitStack()
    wqk_pool = att_stack.enter_context(tc.tile_pool(name="wqk", bufs=1))
    v_pool = att_stack.enter_context(tc.tile_pool(name="v_pool", bufs=1))
    den_pool = att_stack.enter_context(tc.tile_pool(name="den", bufs=1))
    q_pool = att_stack.enter_context(tc.tile_pool(name="q_pool", bufs=2))
    k_pool = att_stack.enter_context(tc.tile_pool(name="k_pool", bufs=2))
    p_pool = att_stack.enter_context(tc.tile_pool(name="p_pool", bufs=8))
    bt_pool = att_stack.enter_context(tc.tile_pool(name="bt_pool", bufs=2))
    dst_pool = att_stack.enter_context(tc.tile_pool(name="dst_pool", bufs=4))
    ps_acc = att_stack.enter_context(
        tc.tile_pool(name="ps_acc", bufs=4, space="PSUM"))

    V_sb = v_pool.tile([P, 8, H, DH + 1], BF16)  # [V | den-col] token-major
    den16 = den_pool.tile([H, TOK], BF16)        # head h denominator on part h

    # ============ phase 0: load x^T + weights, compute V ============
    with tc.tile_pool(name="wv_pool", bufs=1) as wv_pool:
        wv_t = wv_pool.tile([P, 8, D], BF16)
        nc.sync.dma_start(out=wv_t[:, :, 0:TOK], in_=wv3[:, :, 0:TOK])
        for i, tt in enumerate(TT_ORDER):
            eng = nc.sync if i % 2 == 0 else nc.gpsimd
            eng.dma_start(out=X_f[:, :, tt * P:(tt + 1) * P], in_=xt[tt])
        nc.sync.dma_start(out=wv_t[:, :, TOK:], in_=wv3[:, :, TOK:])
        wq_all = wqk_pool.tile([P, 8, 8, P], BF16)
        nc.gpsimd.dma_start(out=wq_all, in_=wq_all_h)
        wk_all = wqk_pool.tile([P, 8, 8, P], BF16)
        nc.gpsimd.dma_start(out=wk_all, in_=wk_all_h)
        load_consts()
        load_params()

        # denominator column (zeroed on prefix tiles for h=0 cores)
        nc.scalar.copy(out=V_sb[:, :, :, DH:DH + 1],
                       in_=vcol.rearrange("p (a b c) -> p a b c", a=8, b=H))

        for tt in TT_ORDER:
            bv_t = bvA if tt < 4 else bvB
            for half in range(2):
                psV = ps_big.tile([P, TOK], F32, tag="ps")
                for dk in range(8):
                    nc.tensor.matmul(psV, X_f[:, dk, tt * P:(tt + 1) * P],
                                     wv_t[:, dk, half * TOK:(half + 1) * TOK],
                                     start=(dk == 0), stop=(dk == 7))
                nc.vector.scalar_tensor_tensor(
                    out=V_sb[:, tt, half * 8:(half + 1) * 8, 0:DH],
                    in0=psV.rearrange("p (h d) -> p h d", d=DH),
                    scalar=0.0, in1=bv_t[:, half * TOK:(half + 1) * TOK]
                    .rearrange("p (h d) -> p h d", d=DH),
                    op0=OP.add, op1=OP.add)

    # wat loads during attention so attn-proj starts immediately after
    wat_pool = ctx.enter_context(tc.tile_pool(name="wat", bufs=1, side="right"))
    wat_all = wat_pool.tile([P, 8, 8, P], BF16)
    nc.scalar.dma_start(out=wat_all, in_=wat_all_h)

    # ============== attention, one head-pair at a time ==============
    for hp in range(8):
        psQ = ps_big.tile([P, TOK], F32, tag="ps")
        for dk in range(8):
            nc.tensor.matmul(psQ, wq_all[:, hp, dk, :], X_f[:, dk, TOK:CTX],
                             start=(dk == 0), stop=(dk == 7))
        q_t = q_pool.tile([P, TOK], BF16, tag="q")
        # fold the 1/sqrt(dh) softmax scale into Q
        nc.vector.tensor_scalar(
            out=q_t, in0=psQ, scalar1=float(1.0 / np.sqrt(DH)),
            scalar2=params["bq_sc"][:, hp:hp + 1], op0=OP.mult, op1=OP.add)

        k_t = k_pool.tile([P, CTX], BF16, tag="k")
        for half in range(2):
            psK = ps_big.tile([P, TOK], F32, tag="ps")
            for dk in range(8):
                nc.tensor.matmul(psK, wk_all[:, hp, dk, :],
                                 X_f[:, dk, half * TOK:(half + 1) * TOK],
                                 start=(dk == 0), stop=(dk == 7))
            nc.vector.tensor_scalar_add(
                out=k_t[:, half * TOK:(half + 1) * TOK], in0=psK,
                scalar1=params["bk_s"][:, hp:hp + 1])

        psA = ps_acc.tile([65, TOK], F32, tag="acc")
        psB = ps_acc.tile([65, TOK], F32, tag="acc")
        for kt in range(8):
            qs = QSTART[kt]
            psSA = ps_big.tile([P, TOK], F32, tag="ps")
            psSB = ps_big.tile([P, TOK], F32, tag="ps")
            nc.tensor.matmul(psSA[:, qs:], k_t[0:64, kt * P:(kt + 1) * P],
                             q_t[0:64, qs:], start=True, stop=True,
                             tile_position=(0, 0))
            nc.tensor.matmul(psSB[:, qs:], k_t[64:128, kt * P:(kt + 1) * P],
                             q_t[64:128, qs:], start=True, stop=True,
                             tile_position=(64, 0))
            pa = p_pool.tile([P, TOK], BF16, tag="p")
            pb = p_pool.tile([P, TOK], BF16, tag="p")
            nc.scalar.activation(pa[:, qs:], psSA[:, qs:], AF.Exp)
            nc.scalar.activation(pb[:, qs:], psSB[:, qs:], AF.Exp)
            if kt >= 4:
                # triangular mask on the diagonal 128-col block only
                nc.vector.tensor_mul(pa[:, qs:qs + P], pa[:, qs:qs + P], tri)
                nc.vector.tensor_mul(pb[:, qs:qs + P], pb[:, qs:qs + P], tri)
            nc.tensor.matmul(psA[:, qs:], V_sb[:, kt, 2 * hp, :], pa[:, qs:],
                             start=(kt == 0), stop=(kt == 7))
            nc.tensor.matmul(psB[:, qs:], V_sb[:, kt, 2 * hp + 1, :],
                             pb[:, qs:], start=(kt == 0), stop=(kt == 7))

        # evict unnormalized numerators + denominator rows (short chain)
        nc.vector.tensor_copy(out=a_all[0:64, hp, :], in_=psA[0:64, :])
        btmp = bt_pool.tile([64, TOK], BF16, tag="bt")
        nc.vector.tensor_copy(out=btmp, in_=psB[0:64, :])
        nc.gpsimd.dma_start(out=a_all[64:128, hp, :], in_=btmp)
        h2 = 2 * hp
        dstgA = dst_pool.tile([65, TOK], BF16, tag="dsta")
        dstgB = dst_pool.tile([65, TOK], BF16, tag="dstb")
        nc.vector.tensor_copy(out=dstgA[64:65, 0:TOK], in_=psA[64:65, :])
        nc.vector.tensor_copy(out=dstgB[64:65, 0:TOK], in_=psB[64:65, :])
        nc.gpsimd.dma_start(
            out=den16[h2:h2 + 1, :], in_=dstgA[64:65, 0:TOK])
        nc.gpsimd.dma_start(
            out=den16[h2 + 1:h2 + 2, :], in_=dstgB[64:65, 0:TOK])

    # batched softmax normalization: recip over all 16 heads at once
    with nc.allow_low_precision(reason="softmax denominators tolerate bf16"):
        nc.vector.reciprocal(out=den16, in_=den16)
    for hp in range(8):
        psRB = ps_big.tile([P, TOK], F32, tag="ps")
        nc.tensor.matmul(psRB, sel16[:, hp, :], den16, start=True, stop=True)
        nc.vector.tensor_mul(a_all[:, hp, :], a_all[:, hp, :], psRB)
    if DEBUG:
        nc.gpsimd.dma_start(out=dbg_a, in_=a_all)
        nc.gpsimd.dma_start(out=dbg_dd, in_=den16)

    att_stack.close()  # wq/wk/V/q/k/p/den/psacc dead

    r1_pool = ctx.enter_context(tc.tile_pool(name="r1_pool", bufs=1,
                                             side="right"))
    r1 = r1_pool.tile([P, 8, TOK], BF16)

    def layer_norm(src, dst, g_s, b_s, ln_ps, ln_sb, psSum, psSq):
        """dst = g * (src - mean) / sqrt(std + eps) + b; stats over features
        (partition direction, 8 tiles). psSum/psSq are pre-accumulated by the
        producer loop. Row math runs full-lane on a DMA-scattered [128, 8]."""
        # row math directly on partition-0 rows: DVE/ACT cost is free-size
        # bound, so [1,512] ops cost the same as [128,4] but skip the DMA
        # scatter/gather roundtrips on the critical path.
        mrow = ln_sb.tile([1, 4, TOK], F32R, tag="mrow")
        nc.vector.tensor_copy(out=mrow[0:1, 2, :], in_=psSum)
        nc.vector.tensor_copy(out=mrow[0:1, 3, :], in_=psSq)
        nc.vector.tensor_scalar_mul(out=mrow[0:1, 0, :], in0=mrow[0:1, 2, :],
                                    scalar1=float(1.0 / D))
        nc.vector.tensor_scalar_mul(out=mrow[0:1, 3, :], in0=mrow[0:1, 3, :],
                                    scalar1=float(1.0 / D))
        nc.vector.tensor_mul(mrow[0:1, 2, :], mrow[0:1, 0, :], mrow[0:1, 0, :])
        nc.vector.tensor_sub(mrow[0:1, 3, :], mrow[0:1, 3, :], mrow[0:1, 2, :])
        # var^-0.25 = exp(-ln(var*c)/4): Ln/Exp/Relu share one ACT table
        nc.scalar.activation(mrow[0:1, 2, :], mrow[0:1, 3, :], AF.Ln,
                             scale=float(D / (D - 1.0)))
        nc.scalar.activation(mrow[0:1, 1, :], mrow[0:1, 2, :], AF.Exp,
                             scale=-0.25)
        psMR = ln_ps.tile([P, 2, TOK], F32, tag="psmr")
        nc.tensor.matmul(psMR[:, 0, :], _r(ones_row), mrow[0:1, 0, :],
                         start=True, stop=True)
        nc.tensor.matmul(psMR[:, 1, :], _r(ones_row), mrow[0:1, 1, :],
                         start=True, stop=True)
        mean_b = ln_sb.tile([P, TOK], BF16, tag="mean_b")
        nc.vector.tensor_copy(out=mean_b, in_=psMR[:, 0, :])
        rs_b = ln_sb.tile([P, TOK], BF16, tag="rs_b")
        nc.vector.tensor_copy(out=rs_b, in_=psMR[:, 1, :])
        for mt in range(8):
            t1 = ln_sb.tile([P, TOK], BF16, tag="t1")
            nc.vector.tensor_sub(t1, src[:, mt, :], mean_b)
            nc.vector.scalar_tensor_tensor(
                out=dst[:, mt, :], in0=t1, scalar=g_s[:, mt:mt + 1],
                in1=rs_b, op0=OP.mult, op1=OP.mult)
            nc.vector.tensor_scalar_add(
                out=dst[:, mt, :], in0=dst[:, mt, :],
                scalar1=b_s[:, mt:mt + 1])

    h1_pool = ctx.enter_context(tc.tile_pool(name="h1_pool", bufs=1))
    h1 = h1_pool.tile([P, 8, TOK], BF16)
    r2y_pool = ctx.enter_context(tc.tile_pool(name="r2y", bufs=1, side="right"))
    r2 = r2y_pool.tile([P, 8, TOK], BF16)
    y = r2y_pool.tile([P, 8, TOK], BF16)

    # ========= attn projection + residual, LN1 stats interleaved =========
    with tc.tile_pool(name="ln1_ps", bufs=1, space="PSUM") as ln1_ps, \
            tc.tile_pool(name="ln1st", bufs=2, space="PSUM") as ln1_st, \
            tc.tile_pool(name="ln1_sb", bufs=2) as ln1_sb:
        psSum = ln1_st.tile([1, TOK], F32, tag="st")
        psSq = ln1_st.tile([1, TOK], F32, tag="st")
        for mt in range(8):
            psO = ps_big.tile([P, TOK], F32, tag="ps")
            for j in range(8):
                nc.tensor.matmul(psO, wat_all[:, mt, j, :], a_all[:, j, :],
                                 start=(j == 0), stop=(j == 7))
            nc.vector.scalar_tensor_tensor(
                out=r1[:, mt, :], in0=psO, scalar=params["battn_s"][:, mt:mt + 1],
                in1=X_f[:, mt, TOK:CTX], op0=OP.add, op1=OP.add)
            sq_t = ln1_sb.tile([P, TOK], BF16, tag="sq")
            nc.vector.tensor_mul(sq_t, r1[:, mt, :], r1[:, mt, :])
            nc.tensor.matmul(psSum, ones_bf, r1[:, mt, :],
                             start=(mt == 0), stop=(mt == 7))
            nc.tensor.matmul(psSq, ones_bf, sq_t,
                             start=(mt == 0), stop=(mt == 7))

        if DEBUG:
            dbg_strow = ln1_sb.tile([1, 2, TOK], F32, tag="dbgrow")
            nc.vector.tensor_copy(out=dbg_strow[0:1, 0, :], in_=psSum)
            nc.vector.tensor_copy(out=dbg_strow[0:1, 1, :], in_=psSq)
            nc.gpsimd.dma_start(out=dbg_st, in_=dbg_strow)
        layer_norm(r1, h1, params["ln1g_s"], params["ln1b_s"], ln1_ps, ln1_sb, psSum, psSq)
        if DEBUG:
            nc.gpsimd.dma_start(out=dbg_r1, in_=r1)
            nc.gpsimd.dma_start(out=dbg_h1, in_=h1)

    # ================= MLP =================
    with tc.tile_pool(name="m1_pool", bufs=1) as m1_pool, \
            tc.tile_pool(name="wfc", bufs=4) as wfc_pool, \
            tc.tile_pool(name="wmlp", bufs=3) as wmlp_pool, \
            tc.tile_pool(name="ln2_ps", bufs=1, space="PSUM") as ln2_ps, \
            tc.tile_pool(name="ln2st", bufs=2, space="PSUM") as ln2_st, \
            tc.tile_pool(name="ln2_sb", bufs=2) as ln2_sb:
        m1 = m1_pool.tile([P, 32, TOK], BF16)
        for mt in range(32):
            wfc_t = wfc_pool.tile([P, 8, P], BF16, tag="wfc")
            nc.gpsimd.dma_start(out=wfc_t, in_=wfc4[mt])
            psF = ps_big.tile([P, TOK], F32, tag="ps")
            for dk in range(8):
                nc.tensor.matmul(psF, wfc_t[:, dk, :], h1[:, dk, :],
                                 start=(dk == 0), stop=(dk == 7))
            nc.scalar.activation(m1[:, mt, :], psF, AF.Relu,
                                 bias=params["bfc_s"][:, mt:mt + 1], scale=1.0)
        psSum2 = ln2_st.tile([1, TOK], F32, tag="st")
        psSq2 = ln2_st.tile([1, TOK], F32, tag="st")
        for mt in range(8):
            wmlp_t = wmlp_pool.tile([P, 32, P], BF16, tag="wmlp")
            nc.gpsimd.dma_start(out=wmlp_t, in_=wmlp4[mt])
            psM = ps_big.tile([P, TOK], F32, tag="ps")
            for k4 in range(32):
                nc.tensor.matmul(psM, wmlp_t[:, k4, :], m1[:, k4, :],
                                 start=(k4 == 0), stop=(k4 == 31))
            nc.vector.scalar_tensor_tensor(
                out=r2[:, mt, :], in0=psM, scalar=params["bmlp_s"][:, mt:mt + 1],
                in1=h1[:, mt, :], op0=OP.add, op1=OP.add)
            sq_t = ln2_sb.tile([P, TOK], BF16, tag="sq")
            nc.vector.tensor_mul(sq_t, r2[:, mt, :], r2[:, mt, :])
            nc.tensor.matmul(psSum2, ones_bf, r2[:, mt, :],
                             start=(mt == 0), stop=(mt == 7))
            nc.tensor.matmul(psSq2, ones_bf, sq_t,
                             start=(mt == 0), stop=(mt == 7))

        y_out = y
        layer_norm(r2, y_out, params["ln2g_s"], params["ln2b_s"], ln2_ps, ln2_sb, psSum2, psSq2)
        for mt in range(8):
            nc.gpsimd.dma_start(out=out[mt], in_=y_out[:, mt, :])

    ctx.close()


_BUILT = None


def _build():
    global _BUILT
    if _BUILT is not None:
        return _BUILT
    nc = bacc.Bacc("TRN2", target_bir_lowering=False, debug=False,
                   enable_asserts=False, num_devices=N_CORES)

    def din(name, shape, dtype=F32):
        return nc.dram_tensor(name, list(shape), dtype, kind="ExternalInput").ap()

    xt = din("xt", (8, P, 8, P), BF16)           # [tt, p, dt, m]
    wq_all = din("wq_all", (P, 8, 8, P), BF16)   # [p, hp, dk, m]
    wk_all = din("wk_all", (P, 8, 8, P), BF16)
    wv3 = din("wv3", (P, 8, D), BF16)            # [p, dk, m]
    b_qkv = din("b_qkv", (3 * D,))
    wat_all = din("wat_all", (P, 8, 8, P), BF16)  # [p, mt, j, m]
    b_attn = din("b_attn_proj", (D,))
    ln1_g = din("ln1_g", (D,))
    ln1_b = din("ln1_b", (D,))
    wfc4 = din("wfc4", (32, P, 8, P), BF16)      # [mt, p, dk, m]
    b_fc = din("b_fc", (4 * D,))
    wmlp4 = din("wmlp4", (8, P, 32, P), BF16)    # [mt, p, k4, m]
    b_mlp = din("b_mlp_proj", (D,))
    ln2_g = din("ln2_g", (D,))
    ln2_b = din("ln2_b", (D,))
    tri = din("tri", (P, P), BF16)               # [m, n] = n >= m
    vcol = din("vcol", (P, 8 * H), BF16)         # denominator column
    bvA = din("bvA", (D,), BF16)                 # v bias, ctx tiles 0..3
    bvB = din("bvB", (D,), BF16)                 # v bias, ctx tiles 4..7
    sel16 = din("sel16", (H, 8, P), BF16)        # head-pair selector
    out_h = nc.dram_tensor("out", [8, P, TOK], BF16, kind="ExternalOutput")
    dbg_a = nc.dram_tensor("dbg_a", [P, 8, TOK], BF16, kind="ExternalOutput")
    dbg_dd = nc.dram_tensor("dbg_dd", [H, TOK], BF16, kind="ExternalOutput")
    dbg_st = nc.dram_tensor("dbg_st", [1, 2, TOK], F32, kind="ExternalOutput")
    dbg_r1 = nc.dram_tensor("dbg_r1", [P, 8, TOK], BF16, kind="ExternalOutput")
    dbg_h1 = nc.dram_tensor("dbg_h1", [P, 8, TOK], BF16, kind="ExternalOutput")

    io = [xt, wq_all, wk_all, wv3, b_qkv, wat_all, b_attn, ln1_g, ln1_b,
          wfc4, b_fc, wmlp4, b_mlp, ln2_g, ln2_b, tri, vcol, bvA, bvB,
          sel16, out_h.ap(), dbg_a.ap(), dbg_dd.ap(),
          dbg_st.ap(), dbg_r1.ap(), dbg_h1.ap()]
    with tile.TileContext(nc) as tc:
        build_block_kernel(nc, tc, io)
    nc.compile()
    _BUILT = nc
    return nc


def _tile4(w, n_in, n_out):
    """[K, M] weight -> [n_out tiles, P, n_in tiles, P]: t4[mt, p, k, m] =
    w[k*P + p, mt*P + m]."""
    K, M = w.shape
    assert K == n_in * P and M == n_out * P
    return np.ascontiguousarray(
        w.reshape(n_in, P, n_out, P).transpose(2, 1, 0, 3))


def _sel16():
    s = np.zeros((H, 8, P), np.float32)
    for hp in range(8):
        s[2 * hp, hp, 0:64] = 1.0
        s[2 * hp + 1, hp, 64:128] = 1.0
    return s


def _wall(w):
    """[K=1024, M=1024] -> [p, mt, k, m] with element = w[k*P+p, mt*P+m]."""
    return np.ascontiguousarray(w.reshape(8, P, 8, P).transpose(1, 2, 0, 3))


def _in_maps(inputs):
    bf = lambda a: np.ascontiguousarray(np.asarray(a, dtype=np.float32)
                                        .astype(ml_dtypes.bfloat16))
    f32 = lambda a: np.ascontiguousarray(np.asarray(a), dtype=np.float32)
    x = np.asarray(inputs["x"], dtype=np.float32)
    w_qkv = np.asarray(inputs["w_qkv"], dtype=np.float32)
    bv = np.asarray(inputs["b_qkv"], dtype=np.float32)[2 * D:]
    shared = {
        "wq_all": bf(_wall(w_qkv[:, 0:D])),
        "wk_all": bf(_wall(w_qkv[:, D:2 * D])),
        "wv3": bf(w_qkv[:, 2 * D:].reshape(8, P, D).transpose(1, 0, 2)),
        "wat_all": bf(_wall(np.asarray(inputs["w_attn_proj"], np.float32))),
        "wfc4": bf(_tile4(np.asarray(inputs["w_fc"], np.float32), 8, 32)),
        "wmlp4": bf(_tile4(np.asarray(inputs["w_mlp_proj"], np.float32), 32, 8)),
        "b_qkv": f32(inputs["b_qkv"]),
        "b_attn_proj": f32(inputs["b_attn_proj"]),
        "ln1_g": f32(inputs["ln1_g"]), "ln1_b": f32(inputs["ln1_b"]),
        "b_fc": f32(inputs["b_fc"]),
        "b_mlp_proj": f32(inputs["b_mlp_proj"]),
        "ln2_g": f32(inputs["ln2_g"]), "ln2_b": f32(inputs["ln2_b"]),
        "tri": bf((np.arange(P)[:, None] <= np.arange(P)[None, :])
                  .astype(np.float32)),
        "sel16": bf(_sel16()),
        "bvB": bf(bv),
    }
    maps = []
    for b in range(B):
        for hh in range(2):
            x_core = np.zeros((CTX, D), np.float32)
            if hh == 1:
                x_core[:TOK] = x[b, :TOK]
            x_core[TOK:] = x[b, hh * TOK:(hh + 1) * TOK]
            xT = x_core.T                      # [D, CTX]
            xt_t = np.ascontiguousarray(       # [tt, p, dt, m]
                xT.reshape(8, P, 8, P).transpose(2, 1, 0, 3))
            vc = np.ones((P, 8, H), np.float32)
            if hh == 0:
                vc[:, 0:4, :] = 0.0
            bvA = bv if hh == 1 else np.zeros_like(bv)
            maps.append({"xt": bf(xt_t), "vcol": bf(vc.reshape(P, 8 * H)),
                         "bvA": bf(bvA), **shared})
    return maps


def run_on_cores(inputs, trace=False, **kwargs):
    """Run the SPMD kernel; returns (full_output, BassKernelResults)."""
    nc = _build()
    maps = _in_maps(inputs)
    res = run_bass_kernel_spmd(nc, maps, core_ids=list(range(N_CORES)),
                               trace=trace, **kwargs)
    out = np.zeros((B, S, D), np.float32)
    for c in range(N_CORES):
        b, hh = divmod(c, 2)
        o = np.asarray(res.results[c]["out"], dtype=np.float32)  # [8, P, TOK]
        out[b, hh * TOK:(hh + 1) * TOK] = o.transpose(2, 0, 1).reshape(TOK, D)
    return out, res


def kernel(**inputs) -> np.ndarray:
    out, _ = run_on_cores(inputs, trace=False)
    return out
